# Initial kernel scaffold
#
"""AvgNeighborSimEncoder kernel for 8 Trainium2 NeuronCores.

Math: for each node, avg over unordered neighbor pairs (m<n) of sim[i_m, i_n]
  = (v^T S v - sum_m S[i_m,i_m]) / 2 / (deg*(deg-1)/2),  v = neighbor count vec.
Then idx = int(avg*1000); out = emb[idx].

Implementation (node-partitioned across 8 cores, no collectives):
  - Core k owns rna nodes [375k, 375(k+1)) and a ~188-node dis shard.
  - Host buckets this core's edges by (slot-tile, 512-col-block); device
    densifies the count matrix A via one-hot matmuls (128 edges/chunk),
    transposes it on the PE to get the matmul operand AT.
  - T = A @ S computed as fp16 hi/lo split of S*2^12 (exact integer counts in
    fp16; |S*2^12 - hi - lo| <= ~2^-10, i.e. fp32-grade after unscaling).
  - quad = rowsum(A * T), diag/deg via a narrow matmul against [diag(S)|1].
  - avg = pair/npair via reciprocal-table gather + 2 exact-residual
    refinements (npair is a small integer); floor via cast+compare fix.
  - emb rows gathered on-device by indirect DMA; host only reassembles shards.
"""
import sys
sys.path.insert(0, "/opt/trn_rl_repo")

import numpy as np

NUM_RNA = 3000
NUM_DIS = 1500
EMB_ROWS = 4500
EMB_DIM = 128
N_CORES = 8

SC = 2.0 ** 12          # plane pre-scale (power of 2: commutes with rounding)
UNSCALE_PAIR = 2.0 ** -13  # 0.5 * 2^-12

RNA_PER_CORE = NUM_RNA // N_CORES      # 375
RNA_SLOT_TILES = 3                     # 375 -> 384 slots
DIS_SLOT_TILES = 2                     # <=188 -> 256 slots
KP_DS = 1536                           # padded ds dim (12 K-tiles)
KP_MS = 3072                           # padded ms dim (24 K-tiles)
KT_DS = KP_DS // 128
KT_MS = KP_MS // 128
NB_DS = KP_DS // 512                   # 3 col blocks
NB_MS = KP_MS // 512                   # 6 col blocks
N_ST = RNA_SLOT_TILES + DIS_SLOT_TILES  # 5 slot tiles total
RECIP_N = 16384

_DIS_SIZES = [188, 188, 188, 188, 187, 187, 187, 187]
_DIS_STARTS = np.concatenate([[0], np.cumsum(_DIS_SIZES)])[:-1]


def _make_planes(S, kp):
    """Pad symmetric sim matrix to [kp,kp], split S*2^12 into fp16 hi+lo."""
    n = S.shape[0]
    Sp = np.zeros((kp, kp), dtype=np.float32)
    Sp[:n, :n] = S
    hi = (Sp * SC).astype(np.float16)
    lo = (Sp * SC - hi.astype(np.float32)).astype(np.float16)
    return hi, lo


def _make_dd(hi, lo):
    """[kp, 2] per plane: col0 = diag of the plane, col1 = ones (hi) / zeros (lo)."""
    kp = hi.shape[0]
    dd_hi = np.zeros((kp, 2), dtype=np.float16)
    dd_lo = np.zeros((kp, 2), dtype=np.float16)
    dd_hi[:, 0] = np.diagonal(hi)
    dd_lo[:, 0] = np.diagonal(lo)
    dd_hi[:, 1] = 1.0
    return dd_hi, dd_lo


def _bucketize(slots, cols, n_st, n_cb):
    """Group edges by (slot//128, col//512). Returns dict bucket -> (slot_loc, col_loc)."""
    st = slots // 128
    cb = cols // 512
    out = {}
    for s in range(n_st):
        for c in range(n_cb):
            m = (st == s) & (cb == c)
            out[(s, c)] = (slots[m] - 128 * s, cols[m] - 512 * c)
    return out


def _pack_edges(per_core_buckets, n_st, n_cb):
    """Unify chunk counts across cores; pack into [128, C] arrays (pad id -1)."""
    schedule = []  # list of (st, cb, n_chunks)
    for s in range(n_st):
        for c in range(n_cb):
            mx = max(len(b[(s, c)][0]) for b in per_core_buckets)
            n_chunks = max(1, -(-mx // 128))
            schedule.append((s, c, n_chunks))
    c_total = sum(n for _, _, n in schedule)
    slot_arrs, col_arrs = [], []
    for b in per_core_buckets:
        sa = np.full((128, c_total), -1, dtype=np.int32)
        ca = np.full((128, c_total), -1, dtype=np.int32)
        off = 0
        for s, c, n_chunks in schedule:
            sl, co = b[(s, c)]
            k = len(sl)
            flat_s = np.full(n_chunks * 128, -1, dtype=np.int32)
            flat_c = np.full(n_chunks * 128, -1, dtype=np.int32)
            flat_s[:k] = sl
            flat_c[:k] = co
            sa[:, off:off + n_chunks] = flat_s.reshape(n_chunks, 128).T
            ca[:, off:off + n_chunks] = flat_c.reshape(n_chunks, 128).T
            off += n_chunks
        slot_arrs.append(sa)
        col_arrs.append(ca)
    return schedule, slot_arrs, col_arrs


def _build_program(sched_rna, c1, sched_dis, c2):
    import concourse.bass as bass
    import concourse.tile as tile
    from concourse import bacc, mybir
    from concourse.masks import make_identity

    f32, f16, i32 = mybir.dt.float32, mybir.dt.float16, mybir.dt.int32
    AOp = mybir.AluOpType

    nc = bacc.Bacc("TRN2", target_bir_lowering=False)

    ds_hi_p = nc.declare_dram_parameter("ds_hi", [KP_DS, KP_DS], f16, isOutput=False)
    ds_lo_p = nc.declare_dram_parameter("ds_lo", [KP_DS, KP_DS], f16, isOutput=False)
    ms_hi_p = nc.declare_dram_parameter("ms_hi", [KP_MS, KP_MS], f16, isOutput=False)
    ms_lo_p = nc.declare_dram_parameter("ms_lo", [KP_MS, KP_MS], f16, isOutput=False)
    ddd_hi_p = nc.declare_dram_parameter("dd_ds_hi", [KP_DS, 2], f16, isOutput=False)
    ddd_lo_p = nc.declare_dram_parameter("dd_ds_lo", [KP_DS, 2], f16, isOutput=False)
    ddm_hi_p = nc.declare_dram_parameter("dd_ms_hi", [KP_MS, 2], f16, isOutput=False)
    ddm_lo_p = nc.declare_dram_parameter("dd_ms_lo", [KP_MS, 2], f16, isOutput=False)
    r_slot_p = nc.declare_dram_parameter("r_slot", [128, c1], i32, isOutput=False)
    r_col_p = nc.declare_dram_parameter("r_col", [128, c1], i32, isOutput=False)
    d_slot_p = nc.declare_dram_parameter("d_slot", [128, c2], i32, isOutput=False)
    d_col_p = nc.declare_dram_parameter("d_col", [128, c2], i32, isOutput=False)
    recip_p = nc.declare_dram_parameter("recip", [RECIP_N, 1], f32, isOutput=False)
    emb_p = nc.declare_dram_parameter("emb", [EMB_ROWS, EMB_DIM], f32, isOutput=False)
    out_p = nc.declare_dram_parameter("out_emb", [N_ST * 128, EMB_DIM], f32, isOutput=True)

    with tile.TileContext(nc) as tc:
        with (
            tc.tile_pool(name="const", bufs=1) as cp,
            tc.tile_pool(name="mats", bufs=1) as mp,
            tc.tile_pool(name="work", bufs=3) as wp,
            tc.tile_pool(name="stream", bufs=3) as sp,
            tc.tile_pool(name="psum_dens", bufs=2, space="PSUM") as pdens,
            tc.tile_pool(name="psum_mm", bufs=4, space="PSUM") as pmm,
        ):
            # ---------- constants ----------
            ident = cp.tile([128, 128], f32)
            make_identity(nc, ident[:])
            iota = cp.tile([128, 512], i32)
            nc.gpsimd.iota(iota[:], pattern=[[1, 512]], base=0, channel_multiplier=0)

            t_rslot = cp.tile([128, c1], i32)
            t_rcol = cp.tile([128, c1], i32)
            t_dslot = cp.tile([128, c2], i32)
            t_dcol = cp.tile([128, c2], i32)
            nc.sync.dma_start(out=t_rslot[:], in_=r_slot_p[:])
            nc.sync.dma_start(out=t_rcol[:], in_=r_col_p[:])
            nc.sync.dma_start(out=t_dslot[:], in_=d_slot_p[:])
            nc.sync.dma_start(out=t_dcol[:], in_=d_col_p[:])

            # ds planes resident [128, KT, KP]
            t_dshi = mp.tile([128, KT_DS, KP_DS], f16)
            t_dslo = mp.tile([128, KT_DS, KP_DS], f16)
            nc.sync.dma_start(out=t_dshi[:], in_=ds_hi_p[:].rearrange("(n p) m -> p n m", p=128))
            nc.sync.dma_start(out=t_dslo[:], in_=ds_lo_p[:].rearrange("(n p) m -> p n m", p=128))
            t_ddd_hi = cp.tile([128, KT_DS, 2], f16)
            t_ddd_lo = cp.tile([128, KT_DS, 2], f16)
            nc.sync.dma_start(out=t_ddd_hi[:], in_=ddd_hi_p[:].rearrange("(n p) m -> p n m", p=128))
            nc.sync.dma_start(out=t_ddd_lo[:], in_=ddd_lo_p[:].rearrange("(n p) m -> p n m", p=128))
            t_ddm_hi = cp.tile([128, KT_MS, 2], f16)
            t_ddm_lo = cp.tile([128, KT_MS, 2], f16)
            nc.sync.dma_start(out=t_ddm_hi[:], in_=ddm_hi_p[:].rearrange("(n p) m -> p n m", p=128))
            nc.sync.dma_start(out=t_ddm_lo[:], in_=ddm_lo_p[:].rearrange("(n p) m -> p n m", p=128))

            # count matrices (elementwise layout) + transposed (matmul lhsT layout)
            t_A = mp.tile([128, RNA_SLOT_TILES, KP_DS], f16)
            t_AT = mp.tile([128, KT_DS, RNA_SLOT_TILES * 128], f16)
            t_B = mp.tile([128, DIS_SLOT_TILES, KP_MS], f16)
            t_BT = mp.tile([128, KT_MS, DIS_SLOT_TILES * 128], f16)

            # ---------- densify via one-hot matmuls ----------
            def densify(schedule, t_slot_ids, t_col_ids, t_dst, t_dstT, n_slot_tiles, n_ktiles):
                off = 0
                for (s, c, n_chunks) in schedule:
                    ps_cnt = pdens.tile([128, 512], f32, space="PSUM", tag="ps_cnt")
                    for j in range(n_chunks):
                        col_idx = off + j
                        oh_r = wp.tile([128, 128], f16, tag="oh_r")
                        oh_d = wp.tile([128, 512], f16, tag="oh_d")
                        nc.vector.tensor_tensor(
                            out=oh_r[:],
                            in0=t_slot_ids[:, col_idx:col_idx + 1].to_broadcast([128, 128]),
                            in1=iota[:, :128], op=AOp.is_equal)
                        nc.vector.tensor_tensor(
                            out=oh_d[:],
                            in0=t_col_ids[:, col_idx:col_idx + 1].to_broadcast([128, 512]),
                            in1=iota[:], op=AOp.is_equal)
                        nc.tensor.matmul(out=ps_cnt[:], lhsT=oh_r[:], rhs=oh_d[:],
                                         start=(j == 0), stop=(j == n_chunks - 1))
                    nc.vector.tensor_copy(out=t_dst[:, s, 512 * c:512 * (c + 1)], in_=ps_cnt[:])
                    off += n_chunks
                # transpose: t_dstT[:, kt, 128*s:128*(s+1)] = t_dst[:, s, 128*kt:...].T
                for s in range(n_slot_tiles):
                    for kt in range(n_ktiles):
                        ps_tr = pdens.tile([128, 128], f32, space="PSUM", tag="ps_tr")
                        nc.tensor.transpose(out=ps_tr[:], in_=t_dst[:, s, 128 * kt:128 * (kt + 1)],
                                            identity=ident[:])
                        nc.vector.tensor_copy(out=t_dstT[:, kt, 128 * s:128 * (s + 1)], in_=ps_tr[:])

            densify(sched_rna, t_rslot, t_rcol, t_A, t_AT, RNA_SLOT_TILES, KT_DS)
            densify(sched_dis, t_dslot, t_dcol, t_B, t_BT, DIS_SLOT_TILES, KT_MS)

            # ---------- per-node accumulators [128, N_ST] ----------
            t_quad = mp.tile([128, N_ST], f32)
            t_diag = mp.tile([128, N_ST], f32)
            t_deg = mp.tile([128, N_ST], f32)
            nc.vector.memset(t_quad[:], 0.0)

            # ---------- rna side: T1 = A @ ds (scaled), quad/diag/deg ----------
            for s in range(RNA_SLOT_TILES):
                for nb in range(NB_DS):
                    ps_t = pmm.tile([128, 512], f32, space="PSUM", tag="ps_t")
                    n_mm = 0
                    for kt in range(KT_DS):
                        for hl, plane in ((0, t_dshi), (1, t_dslo)):
                            nc.tensor.matmul(
                                out=ps_t[:],
                                lhsT=t_AT[:, kt, 128 * s:128 * (s + 1)],
                                rhs=plane[:, kt, 512 * nb:512 * (nb + 1)],
                                start=(n_mm == 0), stop=(n_mm == 2 * KT_DS - 1))
                            n_mm += 1
                    prod = wp.tile([128, 512], f32, tag="prod")
                    nc.vector.tensor_tensor(out=prod[:], in0=ps_t[:],
                                            in1=t_A[:, s, 512 * nb:512 * (nb + 1)], op=AOp.mult)
                    part = wp.tile([128, 1], f32, tag="part")
                    nc.vector.tensor_reduce(out=part[:], in_=prod[:],
                                            axis=mybir.AxisListType.X, op=AOp.add)
                    nc.vector.tensor_tensor(out=t_quad[:, s:s + 1], in0=t_quad[:, s:s + 1],
                                            in1=part[:], op=AOp.add)
                # diag/deg
                ps_dd = pmm.tile([128, 2], f32, space="PSUM", tag="ps_dd")
                n_mm = 0
                for kt in range(KT_DS):
                    for hl, plane in ((0, t_ddd_hi), (1, t_ddd_lo)):
                        nc.tensor.matmul(
                            out=ps_dd[:],
                            lhsT=t_AT[:, kt, 128 * s:128 * (s + 1)],
                            rhs=plane[:, kt, :],
                            start=(n_mm == 0), stop=(n_mm == 2 * KT_DS - 1))
                        n_mm += 1
                nc.vector.tensor_copy(out=t_diag[:, s:s + 1], in_=ps_dd[:, 0:1])
                nc.vector.tensor_copy(out=t_deg[:, s:s + 1], in_=ps_dd[:, 1:2])

            # ---------- dis side: T2 = B @ ms (ms streamed), quad/diag/deg ----------
            for half in range(2):
                cols = slice(half * (KP_MS // 2), (half + 1) * (KP_MS // 2))
                ps_t2 = [pmm.tile([128, 512], f32, space="PSUM", tag=f"ps_t2_{s}_{nb}", bufs=1)
                         for s in range(DIS_SLOT_TILES) for nb in range(NB_MS // 2)]
                n_mm = [0] * len(ps_t2)
                for kt in range(KT_MS):
                    m_hi = sp.tile([128, KP_MS // 2], f16, tag="m_hi")
                    m_lo = sp.tile([128, KP_MS // 2], f16, tag="m_lo")
                    nc.sync.dma_start(out=m_hi[:], in_=ms_hi_p[128 * kt:128 * (kt + 1), cols])
                    nc.sync.dma_start(out=m_lo[:], in_=ms_lo_p[128 * kt:128 * (kt + 1), cols])
                    for s in range(DIS_SLOT_TILES):
                        for nb in range(NB_MS // 2):
                            i = s * (NB_MS // 2) + nb
                            for plane_t in (m_hi, m_lo):
                                nc.tensor.matmul(
                                    out=ps_t2[i][:],
                                    lhsT=t_BT[:, kt, 128 * s:128 * (s + 1)],
                                    rhs=plane_t[:, 512 * nb:512 * (nb + 1)],
                                    start=(n_mm[i] == 0), stop=(n_mm[i] == 2 * KT_MS - 1))
                                n_mm[i] += 1
                for s in range(DIS_SLOT_TILES):
                    for nb in range(NB_MS // 2):
                        i = s * (NB_MS // 2) + nb
                        g_nb = half * (NB_MS // 2) + nb
                        sq = RNA_SLOT_TILES + s
                        prod = wp.tile([128, 512], f32, tag="prod")
                        nc.vector.tensor_tensor(out=prod[:], in0=ps_t2[i][:],
                                                in1=t_B[:, s, 512 * g_nb:512 * (g_nb + 1)],
                                                op=AOp.mult)
                        part = wp.tile([128, 1], f32, tag="part")
                        nc.vector.tensor_reduce(out=part[:], in_=prod[:],
                                                axis=mybir.AxisListType.X, op=AOp.add)
                        nc.vector.tensor_tensor(out=t_quad[:, sq:sq + 1],
                                                in0=t_quad[:, sq:sq + 1],
                                                in1=part[:], op=AOp.add)
            for s in range(DIS_SLOT_TILES):
                sq = RNA_SLOT_TILES + s
                ps_dd = pmm.tile([128, 2], f32, space="PSUM", tag="ps_dd")
                n_mm = 0
                for kt in range(KT_MS):
                    for hl, plane in ((0, t_ddm_hi), (1, t_ddm_lo)):
                        nc.tensor.matmul(
                            out=ps_dd[:],
                            lhsT=t_BT[:, kt, 128 * s:128 * (s + 1)],
                            rhs=plane[:, kt, :],
                            start=(n_mm == 0), stop=(n_mm == 2 * KT_MS - 1))
                        n_mm += 1
                nc.vector.tensor_copy(out=t_diag[:, sq:sq + 1], in_=ps_dd[:, 0:1])
                nc.vector.tensor_copy(out=t_deg[:, sq:sq + 1], in_=ps_dd[:, 1:2])

            # ---------- tail: avg = pair/npair, idx = floor(avg*1000) ----------
            t_pair = mp.tile([128, N_ST], f32)
            nc.vector.tensor_tensor(out=t_pair[:], in0=t_quad[:], in1=t_diag[:], op=AOp.subtract)
            nc.vector.tensor_scalar_mul(t_pair[:], t_pair[:], UNSCALE_PAIR)

            t_np = mp.tile([128, N_ST], f32)
            nc.vector.tensor_tensor(out=t_np[:], in0=t_deg[:], in1=t_deg[:], op=AOp.mult)
            nc.vector.tensor_tensor(out=t_np[:], in0=t_np[:], in1=t_deg[:], op=AOp.subtract)
            nc.vector.tensor_scalar_mul(t_np[:], t_np[:], 0.5)
            nc.vector.tensor_scalar_max(t_np[:], t_np[:], 1.0)

            t_npi = mp.tile([128, N_ST], i32)
            t_npc = mp.tile([128, N_ST], f32)
            nc.vector.tensor_scalar_min(t_npc[:], t_np[:], float(RECIP_N - 1))
            nc.vector.tensor_copy(out=t_npi[:], in_=t_npc[:])

            t_r = mp.tile([128, N_ST], f32)
            for s in range(N_ST):
                nc.gpsimd.indirect_dma_start(
                    out=t_r[:, s:s + 1], out_offset=None, in_=recip_p[:],
                    in_offset=bass.IndirectOffsetOnAxis(ap=t_npi[:, s:s + 1], axis=0))

            # q = pair*r, then two exact-residual refinements
            t_q = mp.tile([128, N_ST], f32)
            t_t = mp.tile([128, N_ST], f32)
            t_e = mp.tile([128, N_ST], f32)
            nc.vector.tensor_tensor(out=t_q[:], in0=t_pair[:], in1=t_r[:], op=AOp.mult)
            for _ in range(2):
                nc.vector.tensor_tensor(out=t_t[:], in0=t_q[:], in1=t_np[:], op=AOp.mult)
                nc.vector.tensor_tensor(out=t_e[:], in0=t_pair[:], in1=t_t[:], op=AOp.subtract)
                nc.vector.tensor_tensor(out=t_e[:], in0=t_e[:], in1=t_r[:], op=AOp.mult)
                nc.vector.tensor_tensor(out=t_q[:], in0=t_q[:], in1=t_e[:], op=AOp.add)

            t_y = mp.tile([128, N_ST], f32)
            nc.vector.tensor_scalar_mul(t_y[:], t_q[:], 1000.0)
            # floor
            t_yi = mp.tile([128, N_ST], i32)
            t_yb = mp.tile([128, N_ST], f32)
            t_g = mp.tile([128, N_ST], f32)
            nc.vector.tensor_copy(out=t_yi[:], in_=t_y[:])
            nc.vector.tensor_copy(out=t_yb[:], in_=t_yi[:])
            nc.vector.tensor_tensor(out=t_g[:], in0=t_yb[:], in1=t_y[:], op=AOp.is_gt)
            nc.vector.tensor_tensor(out=t_yb[:], in0=t_yb[:], in1=t_g[:], op=AOp.subtract)
            t_idx = mp.tile([128, N_ST], i32)
            nc.vector.tensor_copy(out=t_idx[:], in_=t_yb[:])

            # ---------- gather emb rows, write out ----------
            t_out = mp.tile([128, N_ST, EMB_DIM], f32)
            for s in range(N_ST):
                nc.gpsimd.indirect_dma_start(
                    out=t_out[:, s, :], out_offset=None, in_=emb_p[:],
                    in_offset=bass.IndirectOffsetOnAxis(ap=t_idx[:, s:s + 1], axis=0))
            nc.sync.dma_start(out=out_p[:].rearrange("(n p) m -> p n m", p=128), in_=t_out[:])

    nc.compile()
    return nc


def kernel(**inputs):
    from concourse.bass_utils import run_bass_kernel_spmd

    assoc = np.asarray(inputs["associations"], dtype=np.int32)
    ms = np.asarray(inputs["ms"], dtype=np.float32)
    ds = np.asarray(inputs["ds"], dtype=np.float32)
    emb = np.asarray(inputs["emb"], dtype=np.float32)

    rna = assoc[0]
    dis = assoc[1] - NUM_RNA

    ds_hi, ds_lo = _make_planes(ds, KP_DS)
    ms_hi, ms_lo = _make_planes(ms, KP_MS)
    dd_ds_hi, dd_ds_lo = _make_dd(ds_hi, ds_lo)
    dd_ms_hi, dd_ms_lo = _make_dd(ms_hi, ms_lo)

    recip = np.ones((RECIP_N, 1), dtype=np.float32)
    recip[1:, 0] = (1.0 / np.arange(1, RECIP_N, dtype=np.float64)).astype(np.float32)

    rna_buckets, dis_buckets = [], []
    for k in range(N_CORES):
        m1 = (rna // RNA_PER_CORE) == k
        rna_buckets.append(_bucketize(rna[m1] - RNA_PER_CORE * k, dis[m1],
                                      RNA_SLOT_TILES, NB_DS))
        d0 = _DIS_STARTS[k]
        m2 = (dis >= d0) & (dis < d0 + _DIS_SIZES[k])
        dis_buckets.append(_bucketize(dis[m2] - d0, rna[m2], DIS_SLOT_TILES, NB_MS))
    sched_rna, r_slots, r_cols = _pack_edges(rna_buckets, RNA_SLOT_TILES, NB_DS)
    sched_dis, d_slots, d_cols = _pack_edges(dis_buckets, DIS_SLOT_TILES, NB_MS)
    c1 = r_slots[0].shape[1]
    c2 = d_slots[0].shape[1]

    nc = _build_program(sched_rna, c1, sched_dis, c2)

    in_maps = []
    for k in range(N_CORES):
        in_maps.append({
            "ds_hi": ds_hi.view(np.uint16), "ds_lo": ds_lo.view(np.uint16),
            "ms_hi": ms_hi.view(np.uint16), "ms_lo": ms_lo.view(np.uint16),
            "dd_ds_hi": dd_ds_hi.view(np.uint16), "dd_ds_lo": dd_ds_lo.view(np.uint16),
            "dd_ms_hi": dd_ms_hi.view(np.uint16), "dd_ms_lo": dd_ms_lo.view(np.uint16),
            "r_slot": r_slots[k], "r_col": r_cols[k],
            "d_slot": d_slots[k], "d_col": d_cols[k],
            "recip": recip, "emb": emb,
        })

    res = run_bass_kernel_spmd(nc, in_maps, list(range(N_CORES)))

    out = np.empty((EMB_ROWS, EMB_DIM), dtype=np.float32)
    for k in range(N_CORES):
        o = res.results[k]["out_emb"]  # [N_ST*128, EMB_DIM]
        out[RNA_PER_CORE * k: RNA_PER_CORE * (k + 1)] = o[:RNA_PER_CORE]
        d0 = _DIS_STARTS[k]
        nd = _DIS_SIZES[k]
        out[NUM_RNA + d0: NUM_RNA + d0 + nd] = \
            o[RNA_SLOT_TILES * 128: RNA_SLOT_TILES * 128 + nd]
    return out


# revision 17
# speedup vs baseline: 1.4627x; 1.4627x over previous
"""AvgNeighborSimEncoder kernel for 8 Trainium2 NeuronCores.

Math: for each node, avg over unordered neighbor pairs (m<n) of sim[i_m, i_n]
  = (v^T S v - sum_m S[i_m,i_m]) / 2 / (deg*(deg-1)/2),  v = neighbor count vec.
Then idx = int(avg*1000); out = emb[idx].

Implementation (node-partitioned across 8 cores, no collectives):
  - Core k owns rna nodes [375k, 375(k+1)) and a ~188-node dis shard.
  - Host buckets this core's edges by (slot-tile, 512-col-block); device
    densifies the count matrix A via one-hot matmuls (128 edges/chunk),
    transposes it on the PE to get the matmul operand AT.
  - T = A @ S computed as fp16 hi/lo split of S*2^12 (exact integer counts in
    fp16; |S*2^12 - hi - lo| <= ~2^-10, i.e. fp32-grade after unscaling).
  - quad = rowsum(A * T), diag/deg via a narrow matmul against [diag(S)|1].
  - avg = pair/npair via reciprocal-table gather + 2 exact-residual
    refinements (npair is a small integer); floor via cast+compare fix.
  - emb rows gathered on-device by indirect DMA; host only reassembles shards.
"""
import sys
sys.path.insert(0, "/opt/trn_rl_repo")

import numpy as np

NUM_RNA = 3000
NUM_DIS = 1500
EMB_ROWS = 4500
EMB_DIM = 128
N_CORES = 8

SC = 2.0 ** 12          # plane pre-scale (power of 2: commutes with rounding)
UNSCALE_PAIR = 2.0 ** -13  # 0.5 * 2^-12

RNA_PER_CORE = NUM_RNA // N_CORES      # 375
RNA_SLOT_TILES = 3                     # 375 -> 384 slots
DIS_SLOT_TILES = 2                     # <=188 -> 256 slots
KP_DS = 1536                           # padded ds dim (12 K-tiles)
KP_MS = 3072                           # padded ms dim (24 K-tiles)
KT_DS = KP_DS // 128
KT_MS = KP_MS // 128
NB_DS = KP_DS // 512                   # 3 col blocks
NB_MS = KP_MS // 512                   # 6 col blocks
N_ST = RNA_SLOT_TILES + DIS_SLOT_TILES  # 5 slot tiles total
RECIP_N = 16384

_DIS_SIZES = [188, 188, 188, 188, 187, 187, 187, 187]
_DIS_STARTS = np.concatenate([[0], np.cumsum(_DIS_SIZES)])[:-1]


def _make_planes(S, kp):
    """Pad symmetric sim matrix to [kp,kp], split S*2^12 into fp16 hi+lo."""
    n = S.shape[0]
    Sp = np.zeros((kp, kp), dtype=np.float32)
    Sp[:n, :n] = S
    hi = (Sp * SC).astype(np.float16)
    lo = (Sp * SC - hi.astype(np.float32)).astype(np.float16)
    return hi, lo


def _make_dd(hi, lo):
    """[kp, 2] per plane: col0 = diag of the plane, col1 = ones (hi) / zeros (lo)."""
    kp = hi.shape[0]
    dd_hi = np.zeros((kp, 2), dtype=np.float16)
    dd_lo = np.zeros((kp, 2), dtype=np.float16)
    dd_hi[:, 0] = np.diagonal(hi)
    dd_lo[:, 0] = np.diagonal(lo)
    dd_hi[:, 1] = 1.0
    return dd_hi, dd_lo


def _bucketize(slots, cols, n_st, n_cb):
    """Group edges by (slot//128, col//512). Returns dict bucket -> (slot_loc, col_loc)."""
    st = slots // 128
    cb = cols // 512
    out = {}
    for s in range(n_st):
        for c in range(n_cb):
            m = (st == s) & (cb == c)
            out[(s, c)] = (slots[m] - 128 * s, cols[m] - 512 * c)
    return out


def _pack_edges(per_core_buckets, n_st, n_cb):
    """Unify chunk counts across cores; pack into [128, C] arrays (pad id -1)."""
    schedule = []  # list of (st, cb, n_chunks)
    for s in range(n_st):
        for c in range(n_cb):
            mx = max(len(b[(s, c)][0]) for b in per_core_buckets)
            n_chunks = max(1, -(-mx // 128))
            schedule.append((s, c, n_chunks))
    c_total = sum(n for _, _, n in schedule)
    slot_arrs, col_arrs = [], []
    for b in per_core_buckets:
        sa = np.full((128, c_total), -1, dtype=np.int32)
        ca = np.full((128, c_total), -1, dtype=np.int32)
        off = 0
        for s, c, n_chunks in schedule:
            sl, co = b[(s, c)]
            k = len(sl)
            flat_s = np.full(n_chunks * 128, -1, dtype=np.int32)
            flat_c = np.full(n_chunks * 128, -1, dtype=np.int32)
            flat_s[:k] = sl
            flat_c[:k] = co
            sa[:, off:off + n_chunks] = flat_s.reshape(n_chunks, 128).T
            ca[:, off:off + n_chunks] = flat_c.reshape(n_chunks, 128).T
            off += n_chunks
        slot_arrs.append(sa)
        col_arrs.append(ca)
    return schedule, slot_arrs, col_arrs


def _build_program(sched_rna, c1, sched_dis, c2, n_reps=1):
    import concourse.bass as bass
    import concourse.tile as tile
    from concourse import bacc, mybir
    from concourse.masks import make_identity

    f32, f16, i32 = mybir.dt.float32, mybir.dt.float16, mybir.dt.int32
    AOp = mybir.AluOpType

    nc = bacc.Bacc("TRN2", target_bir_lowering=False)

    ds_hi_p = nc.declare_dram_parameter("ds_hi", [KP_DS, KP_DS], f16, isOutput=False)
    ds_lo_p = nc.declare_dram_parameter("ds_lo", [KP_DS, KP_DS], f16, isOutput=False)
    ms_hi_p = nc.declare_dram_parameter("ms_hi", [KP_MS, KP_MS], f16, isOutput=False)
    ms_lo_p = nc.declare_dram_parameter("ms_lo", [KP_MS, KP_MS], f16, isOutput=False)
    ddd_hi_p = nc.declare_dram_parameter("dd_ds_hi", [KP_DS, 2], f16, isOutput=False)
    ddd_lo_p = nc.declare_dram_parameter("dd_ds_lo", [KP_DS, 2], f16, isOutput=False)
    ddm_hi_p = nc.declare_dram_parameter("dd_ms_hi", [KP_MS, 2], f16, isOutput=False)
    ddm_lo_p = nc.declare_dram_parameter("dd_ms_lo", [KP_MS, 2], f16, isOutput=False)
    r_slot_p = nc.declare_dram_parameter("r_slot", [128, c1], i32, isOutput=False)
    r_col_p = nc.declare_dram_parameter("r_col", [128, c1], i32, isOutput=False)
    d_slot_p = nc.declare_dram_parameter("d_slot", [128, c2], i32, isOutput=False)
    d_col_p = nc.declare_dram_parameter("d_col", [128, c2], i32, isOutput=False)
    recip_p = nc.declare_dram_parameter("recip", [RECIP_N, 1], f32, isOutput=False)
    emb_p = nc.declare_dram_parameter("emb", [EMB_ROWS, EMB_DIM], f32, isOutput=False)
    out_p = nc.declare_dram_parameter("out_emb", [N_ST * 128, EMB_DIM], f32, isOutput=True)

    for _rep in range(n_reps):
      with tile.TileContext(nc) as tc:
        with (
            tc.tile_pool(name="const", bufs=1) as cp,
            tc.tile_pool(name="mats", bufs=1) as mp,
            tc.tile_pool(name="work", bufs=3) as wp,
            tc.tile_pool(name="stream", bufs=3) as sp,
        ):
            # ---------- constants ----------
            ident = cp.tile([128, 128], f16)
            make_identity(nc, ident[:])
            iota = cp.tile([128, 512], i32)
            nc.gpsimd.iota(iota[:], pattern=[[1, 512]], base=0, channel_multiplier=0)

            t_rslot = cp.tile([128, c1], i32)
            t_rcol = cp.tile([128, c1], i32)
            t_dslot = cp.tile([128, c2], i32)
            t_dcol = cp.tile([128, c2], i32)
            nc.sync.dma_start(out=t_rslot[:], in_=r_slot_p[:])
            nc.sync.dma_start(out=t_rcol[:], in_=r_col_p[:])
            nc.sync.dma_start(out=t_dslot[:], in_=d_slot_p[:])
            nc.sync.dma_start(out=t_dcol[:], in_=d_col_p[:])

            # ds planes resident [128, KT, KP]
            t_dshi = mp.tile([128, KT_DS, KP_DS], f16)
            t_dslo = mp.tile([128, KT_DS, KP_DS], f16)
            nc.sync.dma_start(out=t_dshi[:], in_=ds_hi_p[:].rearrange("(n p) m -> p n m", p=128))
            nc.sync.dma_start(out=t_dslo[:], in_=ds_lo_p[:].rearrange("(n p) m -> p n m", p=128))
            t_ddd_hi = cp.tile([128, KT_DS, 2], f16)
            t_ddd_lo = cp.tile([128, KT_DS, 2], f16)
            nc.sync.dma_start(out=t_ddd_hi[:], in_=ddd_hi_p[:].rearrange("(n p) m -> p n m", p=128))
            nc.sync.dma_start(out=t_ddd_lo[:], in_=ddd_lo_p[:].rearrange("(n p) m -> p n m", p=128))
            t_ddm_hi = cp.tile([128, KT_MS, 2], f16)
            t_ddm_lo = cp.tile([128, KT_MS, 2], f16)
            nc.sync.dma_start(out=t_ddm_hi[:], in_=ddm_hi_p[:].rearrange("(n p) m -> p n m", p=128))
            nc.sync.dma_start(out=t_ddm_lo[:], in_=ddm_lo_p[:].rearrange("(n p) m -> p n m", p=128))

            # count matrices (elementwise layout) + transposed (matmul lhsT layout)
            t_A = mp.tile([128, RNA_SLOT_TILES, KP_DS], f16)
            t_AT = mp.tile([128, KT_DS, RNA_SLOT_TILES * 128], f16)
            t_B = mp.tile([128, DIS_SLOT_TILES, KP_MS], f16)
            t_BT = mp.tile([128, KT_MS, DIS_SLOT_TILES * 128], f16)

            # ---------- densify via one-hot matmuls ----------
            def densify(pdens, schedule, t_slot_ids, t_col_ids, t_dst, t_dstT,
                        n_slot_tiles, n_ktiles):
                off = 0
                for (s, c, n_chunks) in schedule:
                    ps_cnt = pdens.tile([128, 512], f32, space="PSUM", tag="ps_cnt")
                    for j in range(n_chunks):
                        col_idx = off + j
                        oh_r = wp.tile([128, 128], f16, tag="oh_r")
                        oh_d = wp.tile([128, 512], f16, tag="oh_d")
                        nc.vector.tensor_tensor(
                            out=oh_r[:],
                            in0=t_slot_ids[:, col_idx:col_idx + 1].to_broadcast([128, 128]),
                            in1=iota[:, :128], op=AOp.is_equal)
                        nc.vector.tensor_tensor(
                            out=oh_d[:],
                            in0=t_col_ids[:, col_idx:col_idx + 1].to_broadcast([128, 512]),
                            in1=iota[:], op=AOp.is_equal)
                        nc.tensor.matmul(out=ps_cnt[:], lhsT=oh_r[:], rhs=oh_d[:],
                                         start=(j == 0), stop=(j == n_chunks - 1))
                    nc.vector.tensor_copy(out=t_dst[:, s, 512 * c:512 * (c + 1)], in_=ps_cnt[:])
                    off += n_chunks
                # transpose: t_dstT[:, kt, 128*s:128*(s+1)] = t_dst[:, s, 128*kt:...].T
                for s in range(n_slot_tiles):
                    for kt in range(n_ktiles):
                        ps_tr = pdens.tile([128, 128], f16, space="PSUM", tag="ps_tr")
                        nc.tensor.transpose(out=ps_tr[:], in_=t_dst[:, s, 128 * kt:128 * (kt + 1)],
                                            identity=ident[:])
                        nc.vector.tensor_copy(out=t_dstT[:, kt, 128 * s:128 * (s + 1)], in_=ps_tr[:])

            with tc.tile_pool(name="psum_dens", bufs=2, space="PSUM") as pdens:
                densify(pdens, sched_rna, t_rslot, t_rcol, t_A, t_AT, RNA_SLOT_TILES, KT_DS)
                densify(pdens, sched_dis, t_dslot, t_dcol, t_B, t_BT, DIS_SLOT_TILES, KT_MS)

            # ---------- per-node accumulators [128, N_ST] ----------
            t_quad = mp.tile([128, N_ST], f32)
            t_diag = mp.tile([128, N_ST], f32)
            t_deg = mp.tile([128, N_ST], f32)
            nc.vector.memset(t_quad[:], 0.0)

            # ---------- rna side: T1 = A @ ds (scaled), quad/diag/deg ----------
            prna_cm = tc.tile_pool(name="psum_rna", bufs=2, space="PSUM")
            prna = prna_cm.__enter__()
            for s in range(RNA_SLOT_TILES):
                for nb in range(NB_DS):
                    ps_t = prna.tile([128, 512], f32, space="PSUM", tag="ps_t")
                    n_mm = 0
                    for kt in range(KT_DS):
                        for hl, plane in ((0, t_dshi), (1, t_dslo)):
                            nc.tensor.matmul(
                                out=ps_t[:],
                                lhsT=t_AT[:, kt, 128 * s:128 * (s + 1)],
                                rhs=plane[:, kt, 512 * nb:512 * (nb + 1)],
                                start=(n_mm == 0), stop=(n_mm == 2 * KT_DS - 1))
                            n_mm += 1
                    prod = wp.tile([128, 512], f32, tag="prod")
                    nc.vector.tensor_tensor(out=prod[:], in0=ps_t[:],
                                            in1=t_A[:, s, 512 * nb:512 * (nb + 1)], op=AOp.mult)
                    part = wp.tile([128, 1], f32, tag="part")
                    nc.vector.tensor_reduce(out=part[:], in_=prod[:],
                                            axis=mybir.AxisListType.X, op=AOp.add)
                    nc.vector.tensor_tensor(out=t_quad[:, s:s + 1], in0=t_quad[:, s:s + 1],
                                            in1=part[:], op=AOp.add)
                # diag/deg
                ps_dd = prna.tile([128, 2], f32, space="PSUM", tag="ps_dd")
                n_mm = 0
                for kt in range(KT_DS):
                    for hl, plane in ((0, t_ddd_hi), (1, t_ddd_lo)):
                        nc.tensor.matmul(
                            out=ps_dd[:],
                            lhsT=t_AT[:, kt, 128 * s:128 * (s + 1)],
                            rhs=plane[:, kt, :],
                            start=(n_mm == 0), stop=(n_mm == 2 * KT_DS - 1))
                        n_mm += 1
                nc.vector.tensor_copy(out=t_diag[:, s:s + 1], in_=ps_dd[:, 0:1])
                nc.vector.tensor_copy(out=t_deg[:, s:s + 1], in_=ps_dd[:, 1:2])
            prna_cm.__exit__(None, None, None)

            # ---------- dis side: T2 = B @ ms (ms streamed), quad/diag/deg ----------
            pdis_cm = tc.tile_pool(name="psum_dis", bufs=1, space="PSUM")
            pdis = pdis_cm.__enter__()
            for half in range(2):
                cols = slice(half * (KP_MS // 2), (half + 1) * (KP_MS // 2))
                ps_t2 = [pdis.tile([128, 512], f32, space="PSUM",
                                   tag=f"ps_t2_{s}_{nb}", name=f"ps_t2_{s}_{nb}", bufs=1)
                         for s in range(DIS_SLOT_TILES) for nb in range(NB_MS // 2)]
                n_mm = [0] * len(ps_t2)
                for kt in range(KT_MS):
                    m_hi = sp.tile([128, KP_MS // 2], f16, tag="m_hi")
                    m_lo = sp.tile([128, KP_MS // 2], f16, tag="m_lo")
                    nc.sync.dma_start(out=m_hi[:], in_=ms_hi_p[128 * kt:128 * (kt + 1), cols])
                    nc.sync.dma_start(out=m_lo[:], in_=ms_lo_p[128 * kt:128 * (kt + 1), cols])
                    for s in range(DIS_SLOT_TILES):
                        for nb in range(NB_MS // 2):
                            i = s * (NB_MS // 2) + nb
                            for plane_t in (m_hi, m_lo):
                                nc.tensor.matmul(
                                    out=ps_t2[i][:],
                                    lhsT=t_BT[:, kt, 128 * s:128 * (s + 1)],
                                    rhs=plane_t[:, 512 * nb:512 * (nb + 1)],
                                    start=(n_mm[i] == 0), stop=(n_mm[i] == 2 * KT_MS - 1))
                                n_mm[i] += 1
                for s in range(DIS_SLOT_TILES):
                    for nb in range(NB_MS // 2):
                        i = s * (NB_MS // 2) + nb
                        g_nb = half * (NB_MS // 2) + nb
                        sq = RNA_SLOT_TILES + s
                        prod = wp.tile([128, 512], f32, tag="prod")
                        nc.vector.tensor_tensor(out=prod[:], in0=ps_t2[i][:],
                                                in1=t_B[:, s, 512 * g_nb:512 * (g_nb + 1)],
                                                op=AOp.mult)
                        part = wp.tile([128, 1], f32, tag="part")
                        nc.vector.tensor_reduce(out=part[:], in_=prod[:],
                                                axis=mybir.AxisListType.X, op=AOp.add)
                        nc.vector.tensor_tensor(out=t_quad[:, sq:sq + 1],
                                                in0=t_quad[:, sq:sq + 1],
                                                in1=part[:], op=AOp.add)
            for s in range(DIS_SLOT_TILES):
                sq = RNA_SLOT_TILES + s
                ps_dd = pdis.tile([128, 2], f32, space="PSUM", tag="ps_dd")
                n_mm = 0
                for kt in range(KT_MS):
                    for hl, plane in ((0, t_ddm_hi), (1, t_ddm_lo)):
                        nc.tensor.matmul(
                            out=ps_dd[:],
                            lhsT=t_BT[:, kt, 128 * s:128 * (s + 1)],
                            rhs=plane[:, kt, :],
                            start=(n_mm == 0), stop=(n_mm == 2 * KT_MS - 1))
                        n_mm += 1
                nc.vector.tensor_copy(out=t_diag[:, sq:sq + 1], in_=ps_dd[:, 0:1])
                nc.vector.tensor_copy(out=t_deg[:, sq:sq + 1], in_=ps_dd[:, 1:2])
            pdis_cm.__exit__(None, None, None)

            # ---------- tail: avg = pair/npair, idx = floor(avg*1000) ----------
            t_pair = mp.tile([128, N_ST], f32)
            nc.vector.tensor_tensor(out=t_pair[:], in0=t_quad[:], in1=t_diag[:], op=AOp.subtract)
            nc.vector.tensor_scalar_mul(t_pair[:], t_pair[:], UNSCALE_PAIR)

            t_np = mp.tile([128, N_ST], f32)
            nc.vector.tensor_tensor(out=t_np[:], in0=t_deg[:], in1=t_deg[:], op=AOp.mult)
            nc.vector.tensor_tensor(out=t_np[:], in0=t_np[:], in1=t_deg[:], op=AOp.subtract)
            nc.vector.tensor_scalar_mul(t_np[:], t_np[:], 0.5)
            nc.vector.tensor_scalar_max(t_np[:], t_np[:], 1.0)

            t_npi = mp.tile([128, N_ST], i32)
            t_npc = mp.tile([128, N_ST], f32)
            nc.vector.tensor_scalar_min(t_npc[:], t_np[:], float(RECIP_N - 1))
            nc.vector.tensor_copy(out=t_npi[:], in_=t_npc[:])

            t_r = mp.tile([128, N_ST], f32)
            for s in range(N_ST):
                nc.gpsimd.indirect_dma_start(
                    out=t_r[:, s:s + 1], out_offset=None, in_=recip_p[:],
                    in_offset=bass.IndirectOffsetOnAxis(ap=t_npi[:, s:s + 1], axis=0))

            # q = pair*r, then two exact-residual refinements
            t_q = mp.tile([128, N_ST], f32)
            t_t = mp.tile([128, N_ST], f32)
            t_e = mp.tile([128, N_ST], f32)
            nc.vector.tensor_tensor(out=t_q[:], in0=t_pair[:], in1=t_r[:], op=AOp.mult)
            for _ in range(2):
                nc.vector.tensor_tensor(out=t_t[:], in0=t_q[:], in1=t_np[:], op=AOp.mult)
                nc.vector.tensor_tensor(out=t_e[:], in0=t_pair[:], in1=t_t[:], op=AOp.subtract)
                nc.vector.tensor_tensor(out=t_e[:], in0=t_e[:], in1=t_r[:], op=AOp.mult)
                nc.vector.tensor_tensor(out=t_q[:], in0=t_q[:], in1=t_e[:], op=AOp.add)

            t_y = mp.tile([128, N_ST], f32)
            nc.vector.tensor_scalar_mul(t_y[:], t_q[:], 1000.0)
            nc.vector.tensor_scalar_max(t_y[:], t_y[:], 0.0)
            # int cast: round-to-nearest, matching the on-device XLA cast the
            # reference lowers to on this backend (not C truncation).
            t_idx = mp.tile([128, N_ST], i32)
            nc.vector.tensor_copy(out=t_idx[:], in_=t_y[:])

            # ---------- gather emb rows, write out ----------
            t_out = mp.tile([128, N_ST, EMB_DIM], f32)
            for s in range(N_ST):
                nc.gpsimd.indirect_dma_start(
                    out=t_out[:, s, :], out_offset=None, in_=emb_p[:],
                    in_offset=bass.IndirectOffsetOnAxis(ap=t_idx[:, s:s + 1], axis=0))
            nc.sync.dma_start(out=out_p[:].rearrange("(n p) m -> p n m", p=128), in_=t_out[:])

    nc.compile()
    return nc


def _prepare(inputs, n_reps=1):
    assoc = np.asarray(inputs["associations"], dtype=np.int32)
    ms = np.asarray(inputs["ms"], dtype=np.float32)
    ds = np.asarray(inputs["ds"], dtype=np.float32)
    emb = np.asarray(inputs["emb"], dtype=np.float32)

    rna = assoc[0]
    dis = assoc[1] - NUM_RNA

    ds_hi, ds_lo = _make_planes(ds, KP_DS)
    ms_hi, ms_lo = _make_planes(ms, KP_MS)
    dd_ds_hi, dd_ds_lo = _make_dd(ds_hi, ds_lo)
    dd_ms_hi, dd_ms_lo = _make_dd(ms_hi, ms_lo)

    recip = np.ones((RECIP_N, 1), dtype=np.float32)
    recip[1:, 0] = (1.0 / np.arange(1, RECIP_N, dtype=np.float64)).astype(np.float32)

    rna_buckets, dis_buckets = [], []
    for k in range(N_CORES):
        m1 = (rna // RNA_PER_CORE) == k
        rna_buckets.append(_bucketize(rna[m1] - RNA_PER_CORE * k, dis[m1],
                                      RNA_SLOT_TILES, NB_DS))
        d0 = _DIS_STARTS[k]
        m2 = (dis >= d0) & (dis < d0 + _DIS_SIZES[k])
        dis_buckets.append(_bucketize(dis[m2] - d0, rna[m2], DIS_SLOT_TILES, NB_MS))
    sched_rna, r_slots, r_cols = _pack_edges(rna_buckets, RNA_SLOT_TILES, NB_DS)
    sched_dis, d_slots, d_cols = _pack_edges(dis_buckets, DIS_SLOT_TILES, NB_MS)
    c1 = r_slots[0].shape[1]
    c2 = d_slots[0].shape[1]

    nc = _build_program(sched_rna, c1, sched_dis, c2, n_reps=n_reps)

    in_maps = []
    for k in range(N_CORES):
        in_maps.append({
            "ds_hi": ds_hi.view(np.uint16), "ds_lo": ds_lo.view(np.uint16),
            "ms_hi": ms_hi.view(np.uint16), "ms_lo": ms_lo.view(np.uint16),
            "dd_ds_hi": dd_ds_hi.view(np.uint16), "dd_ds_lo": dd_ds_lo.view(np.uint16),
            "dd_ms_hi": dd_ms_hi.view(np.uint16), "dd_ms_lo": dd_ms_lo.view(np.uint16),
            "r_slot": r_slots[k], "r_col": r_cols[k],
            "d_slot": d_slots[k], "d_col": d_cols[k],
            "recip": recip, "emb": emb,
        })

    return nc, in_maps


def _unshard(results):
    out = np.empty((EMB_ROWS, EMB_DIM), dtype=np.float32)
    for k in range(N_CORES):
        o = results[k]["out_emb"]  # [N_ST*128, EMB_DIM]
        out[RNA_PER_CORE * k: RNA_PER_CORE * (k + 1)] = o[:RNA_PER_CORE]
        d0 = _DIS_STARTS[k]
        nd = _DIS_SIZES[k]
        out[NUM_RNA + d0: NUM_RNA + d0 + nd] = \
            o[RNA_SLOT_TILES * 128: RNA_SLOT_TILES * 128 + nd]
    return out


def kernel(**inputs):
    from concourse.bass_utils import run_bass_kernel_spmd
    nc, in_maps = _prepare(inputs)
    res = run_bass_kernel_spmd(nc, in_maps, list(range(N_CORES)))
    return _unshard(res.results)


def profile_exec_ns(inputs):
    """Profiled run (test-only helper): returns max per-core HW exec time in ns."""
    from concourse.bass_utils import run_bass_kernel_spmd
    nc, in_maps = _prepare(inputs)
    res = run_bass_kernel_spmd(nc, in_maps, list(range(N_CORES)),
                               trace=True, trace_cores=list(range(N_CORES)))
    return res.exec_time_ns
